# revision 1
# baseline (speedup 1.0000x reference)
"""GCNConv Trainium2 kernel: 8-way destination-node sharding.

out = A_norm @ x @ W.T + b, A_norm sparse (~650k nnz incl. self loops).

v3 (default): per core, 10 dst tiles of 128 nodes; src space in 79 chunks of
128. The sparse A is consumed as dense 128x128 blocks C[src_local, dst_local]
built on-chip right before use, one "C-row" [128, 10*128] per src chunk,
accumulated into a 1280-wide fp32 PSUM aggregate by one LDW + few matmuls
per chunk (lhsT = resident x chunk). The 10 tiles of each C-row are built by
three different producers, balancing all engines:
  - tiles 0..NVT-1 (DVE): host pre-assigns <=1 edge per (src, tile); one
    fused tensor_scalar per cell: C = (iota==dst_slot) * norm_slot.
    Edges beyond the first per (src, tile) spill to the overflow pass.
  - tiles NVT..NDT-1 (host-streamed): full dense blocks built on host
    (numpy scatter of norms; holds ALL edges) and DMA-streamed like x.
  - tiles NDT..9 (GPSIMD): one local_scatter per chunk builds all 3 tiles
    from per-partition (idx, val) lists; holds ALL edges, no overflow.
Overflow edges (2nd+ edge per (src, dve-tile), ~12% of edges) are bucketed
per tile into 128-edge blocks; the host pre-gathers their x rows (numpy
indexing -> xovf stream), DVE builds the dst one-hot M per block, and PE
accumulates xovf.T @ M into the tile's PSUM columns, interleaved into the
chunk loop (window chunks OVS..OVE<78 so all writes precede the stop-marked
final-chunk matmuls). Tail: ACT copies aggregate->SBUF fp16 (compact), PE
applies W (lhsT=W.T stationary), DVE adds bias per partition, DMA out as
[d, tile*128+n] (host transposes).
"""

import os
import numpy as np

N_NODES = 10000
D = 128
P = 128
N_CORES = 8
TILES_PER_CORE = 10  # 8 cores * 10 tiles * 128 = 10240 slots >= 10000
N_TILES_TOTAL = N_CORES * TILES_PER_CORE

_cache = {}


def _build_program(B, use_gates=True, repeat=1):
    import concourse.bass as bass
    import concourse.bacc as bacc
    import concourse.mybir as mybir
    import concourse.tile as tile
    from contextlib import ExitStack

    nt = TILES_PER_CORE
    fp32 = mybir.dt.float32

    nc = bacc.Bacc(
        "TRN2", target_bir_lowering=False, debug=False, num_devices=N_CORES
    )
    ncols = 2 * nt * B + 3 * P  # dstl | nrm | iota | bb | wt
    x_d = nc.dram_tensor("x", [N_NODES, D], fp32, kind="ExternalInput")
    meta_d = nc.dram_tensor("meta", [P, ncols], fp32, kind="ExternalInput")
    idx_d = nc.dram_tensor(
        "idx", [P, nt * B * 8], mybir.dt.int16, kind="ExternalInput"
    )
    # one output tensor per tile: avoids coarse per-DRAM-tensor WAW waits on
    # the output DMAs (walrus allows only one sync wait per DMA)
    out_ds = [
        nc.dram_tensor(f"out{t}", [P, D], fp32, kind="ExternalOutput")
        for t in range(nt)
    ]

    from concourse.tile import add_dep_helper
    from concourse import library_config

    with tile.TileContext(nc) as tc, ExitStack() as ctx:
        cpool = ctx.enter_context(tc.tile_pool(name="const", bufs=1))
        gpool = ctx.enter_context(tc.tile_pool(name="gather", bufs=2))
        mpool = ctx.enter_context(tc.tile_pool(name="onehot", bufs=8))
        opool = ctx.enter_context(tc.tile_pool(name="outs", bufs=12))
        pspool = ctx.enter_context(tc.tile_pool(name="psum", bufs=4, space="PSUM"))

        hw_hist = []  # HWDGE DMAs issued from SP, in order (8 sem lanes)
        meta_sb = cpool.tile([P, ncols], fp32)
        meta_load = nc.sync.dma_start(meta_sb[:], meta_d[:, :])
        hw_hist.append(meta_load)
        dstl_sb = meta_sb[:, 0 : nt * B]
        nrm_sb = meta_sb[:, nt * B : 2 * nt * B]
        iota_sb = meta_sb[:, 2 * nt * B : 2 * nt * B + P]
        bb_sb = meta_sb[:, 2 * nt * B + P : 2 * nt * B + 2 * P]
        wt_sb = meta_sb[:, 2 * nt * B + 2 * P : 2 * nt * B + 3 * P]
        idx_sb = cpool.tile([P, nt * B * 8], mybir.dt.int16)
        hw_hist.append(nc.sync.dma_start(idx_sb[:], idx_d[:, :]))

        nc.gpsimd.load_library(library_config.mlp)

        # PE observes the meta load once, so matmuls reading wt need no wait
        meta_gate = nc.tensor.nop(hint="dep")
        add_dep_helper(meta_gate.ins, meta_load.ins, reason="PE observes meta")

        # one dma_gather is limited to 1024 descriptors by the SWDGE ring
        GC = 8  # blocks (128 idxs each) per gather chunk
        NG = (B + GC - 1) // GC
        act_hist = []
        tt_hist = []
        gather_hist = []  # per tile: list of chunk gathers
        ggate_hist = []  # per tile: list of PE chunk gates
        lastmm_hist = []
        wmm_hist = []
        pe_gate_hist = [meta_gate]
        for it in range(repeat * nt):
            t = it % nt
            if it >= 2:
                # release deps of the g slot being rewritten, each absorbed by
                # a single-wait Pool nop (same-engine order gates the gather)
                for dep in (
                    gather_hist[it - 2]
                    + [lastmm_hist[it - 2]]
                    + ggate_hist[it - 2]
                ):
                    pg = nc.gpsimd.nop(hint="dep")
                    add_dep_helper(pg.ins, dep.ins, reason="g slot release")
            g = gpool.tile([P, B, D], fp32)
            chunk_gathers = []
            chunk_gates = []
            for sc in range(NG):
                nb = min(GC, B - sc * GC)
                ni = nb * P
                c0 = t * B * 8 + sc * GC * 8
                gth = nc.gpsimd.dma_gather(
                    g[:, sc * GC : sc * GC + nb, :],
                    x_d[:, :],
                    idx_sb[:, c0 : c0 + nb * 8],
                    ni,
                    ni,
                    D,
                )
                chunk_gathers.append(gth)
            gather_hist.append(chunk_gathers)
            if it >= 4:
                # absorb PSUM bank-release waits (pool bufs=4) into PE nops
                ps_gate = nc.tensor.nop(hint="dep")
                add_dep_helper(
                    ps_gate.ins, act_hist[it - 4].ins, reason="aggT bank free"
                )
                ps_gate2 = nc.tensor.nop(hint="dep")
                add_dep_helper(
                    ps_gate2.ins, tt_hist[it - 4].ins, reason="out_ps bank free"
                )
                pe_gate_hist.extend([ps_gate, ps_gate2])
            aggT_ps = pspool.tile([P, P], fp32)
            for j in range(B):
                if j % GC == 0:
                    # PE observes this chunk's gather (walrus allows only one
                    # sync wait per matmul, so matmuls must not wait on both
                    # the gather DMA and the DVE one-hot build)
                    g_gate = nc.tensor.nop(hint="dep")
                    add_dep_helper(
                        g_gate.ins,
                        chunk_gathers[j // GC].ins,
                        reason="PE observes gather",
                    )
                    chunk_gates.append(g_gate)
                    pe_gate_hist.append(g_gate)
                col = t * B + j
                m = mpool.tile([P, P], fp32)
                nc.vector.tensor_scalar(
                    m[:],
                    iota_sb,
                    dstl_sb[:, col : col + 1],
                    nrm_sb[:, col : col + 1],
                    mybir.AluOpType.is_equal,
                    mybir.AluOpType.mult,
                )
                mm = nc.tensor.matmul(
                    aggT_ps[:],
                    lhsT=g[:, j, :],
                    rhs=m[:],
                    start=(j == 0),
                    stop=(j == B - 1),
                )
                if j == B - 1:
                    lastmm_hist.append(mm)
            ggate_hist.append(chunk_gates)
            aggT_sb = opool.tile([P, P], fp32)
            act = nc.scalar.activation(
                aggT_sb[:], aggT_ps[:], mybir.ActivationFunctionType.Copy
            )
            act_hist.append(act)
            out_ps = pspool.tile([P, P], fp32)
            wmm = nc.tensor.matmul(
                out_ps[:], lhsT=aggT_sb[:], rhs=wt_sb, start=True, stop=True
            )
            wmm_hist.append(wmm)
            out_sb = opool.tile([P, D], fp32)
            tt = nc.vector.tensor_tensor(
                out_sb[:], out_ps[:], bb_sb, op=mybir.AluOpType.add
            )
            tt_hist.append(tt)
            if len(hw_hist) >= 8:
                # absorb the HWDGE sem-lane recycling wait into an SP nop
                sp_gate = nc.sync.nop(hint="dep")
                add_dep_helper(
                    sp_gate.ins, hw_hist[-8].ins, reason="HWDGE lane recycle"
                )
            hw_hist.append(nc.sync.dma_start(out_ds[t][:, :], out_sb[:]))

        # Tail chain: SP observes every outstanding completion through
        # single-wait nops, so the kernel-tail drain's waits all dedup away
        # (the drain's CTRL_NO struct also allows only one sync wait).
        tail_deps = (
            hw_hist
            + [gg for gl in gather_hist[2:] for gg in gl]
            + [tt_hist[-1], act_hist[-1], wmm_hist[-1], lastmm_hist[-1]]
            + pe_gate_hist[-4:]
        )
        for dep in tail_deps:
            tn = nc.sync.nop(hint="dep")
            add_dep_helper(tn.ins, dep.ins, reason="tail drain observe")

    nc.compile()
    if use_gates:
        _dedup_waits(nc)
    return nc


def _dedup_waits(nc):
    """Strip semaphore waits that are provably redundant:
    - a wait already covered by an earlier wait on the same engine queue
      (engines dispatch in order, so a later duplicate is redundant);
    - a wait by engine E on E's own completion semaphore for a value that
      prior E-instructions already incremented to (compute engines complete
      in order).
    The neuronx-cc walrus allows only 1 sync wait per engine instruction,
    and Tile's sem assignment is not transitively minimal, so the gate
    nops' waits must be deduplicated off the real instructions. Barrier
    semaphores (which are decremented) are never touched."""
    fn = nc.m.functions[0]
    for blk in fn.blocks:
        observed = {}  # engine -> {sem_name: max_waited_value}
        self_incs = {}  # sem_name -> total increments seen so far
        for inst in blk.instructions:
            si = getattr(inst, "sync_info", None)
            eng = getattr(inst, "engine", None)
            if si is None or eng is None:
                continue
            ename = str(eng).split(".")[-1]
            w = getattr(si, "on_wait", None) or []
            if w:
                seen = observed.setdefault(ename, {})
                kept = []
                changed = False
                for x in w:
                    if (
                        getattr(x, "wait_mode", None) != "sem-ge-imm"
                        or getattr(x, "wait_value", None) is None
                        or "barrier" in x.ant_name
                    ):
                        kept.append(x)
                        continue
                    prev = seen.get(x.ant_name)
                    if prev is not None and prev >= x.wait_value:
                        changed = True
                        continue
                    sem_owner = x.ant_name.rsplit("_", 1)[0]
                    if (
                        sem_owner == ename
                        and self_incs.get(x.ant_name, 0) >= x.wait_value
                    ):
                        changed = True
                        continue
                    kept.append(x)
                    seen[x.ant_name] = x.wait_value
                if changed:
                    si.on_wait = kept
            for u in getattr(si, "on_update", None) or []:
                name = getattr(u, "ant_name", None)
                val = getattr(u, "update_value", None) or 0
                mode = getattr(u, "update_mode", "")
                if name is not None and "barrier" not in name and "inc" in str(mode):
                    self_incs[name] = self_incs.get(name, 0) + val


def _prep(x, W, b, edge_weight, edge_index):
    src = edge_index[0].astype(np.int64)
    dst = edge_index[1].astype(np.int64)
    ew = edge_weight.astype(np.float32)
    loops = np.arange(N_NODES, dtype=np.int64)
    src = np.concatenate([src, loops])
    dst = np.concatenate([dst, loops])
    ew = np.concatenate([ew, np.ones(N_NODES, np.float32)])

    deg = np.bincount(dst, weights=ew, minlength=N_NODES)
    dinv = np.zeros(N_NODES, np.float64)
    pos = deg > 0
    dinv[pos] = 1.0 / np.sqrt(deg[pos])
    dinv = dinv.astype(np.float32)
    norm = (dinv[src] * ew * dinv[dst]).astype(np.float32)

    order = np.argsort(dst, kind="stable")
    src, dst, norm = src[order], dst[order], norm[order]

    g_tile = dst // P  # global tile id, 0..78
    counts = np.bincount(g_tile, minlength=N_TILES_TOTAL)
    B = int(np.ceil(counts.max() / P))
    # rank of each edge within its tile
    tile_starts = np.zeros(N_TILES_TOTAL + 1, np.int64)
    np.cumsum(counts, out=tile_starts[1:])
    q = np.arange(len(dst)) - tile_starts[g_tile]

    # flat slot: core c, tile t, block j=q//P, partition p=q%P
    c = g_tile // TILES_PER_CORE
    t = g_tile % TILES_PER_CORE
    j = q // P
    p = q % P
    slot = ((c * TILES_PER_CORE + t) * B + j) * P + p

    nslots = N_CORES * TILES_PER_CORE * B * P
    idx_flat = np.zeros(nslots, np.int32)
    dstl_flat = np.zeros(nslots, np.float32)
    nrm_flat = np.zeros(nslots, np.float32)
    idx_flat[slot] = src
    dstl_flat[slot] = (dst - g_tile * P).astype(np.float32)
    nrm_flat[slot] = norm

    # [cores, nt*B, P] -> [cores, P, nt*B]
    shape = (N_CORES, TILES_PER_CORE * B, P)
    dstl_pc = dstl_flat.reshape(shape).transpose(0, 2, 1)
    nrm_pc = nrm_flat.reshape(shape).transpose(0, 2, 1)
    # dma_gather index layout: per (core, tile, chunk of GC=8 blocks), the
    # chunk's indices in flat order (j*128+p) wrapped column-major into 16
    # partitions (block[r, c] = flat[c*16 + r]), replicated over the 8 Q7
    # core groups -> [128, nt*B*8] per core.
    GC = 8
    idx3 = idx_flat.astype(np.int16).reshape(N_CORES, TILES_PER_CORE, B * P)
    idx_rows = np.zeros((N_CORES, 16, TILES_PER_CORE * B * 8), np.int16)
    for t in range(TILES_PER_CORE):
        for sc in range((B + GC - 1) // GC):
            nb = min(GC, B - sc * GC)
            seg = idx3[:, t, sc * GC * P : sc * GC * P + nb * P]
            blk = seg.reshape(N_CORES, nb * 8, 16).swapaxes(1, 2)
            c0 = t * B * 8 + sc * GC * 8
            idx_rows[:, :, c0 : c0 + nb * 8] = blk
    idx_pc = np.ascontiguousarray(np.tile(idx_rows, (1, 8, 1)))

    wt = np.ascontiguousarray(W.T.astype(np.float32))
    bb = np.tile(b.astype(np.float32)[None, :], (P, 1))
    iota = np.tile(np.arange(P, dtype=np.float32)[None, :], (P, 1))
    x_full = np.ascontiguousarray(x, dtype=np.float32)

    in_maps = []
    for core in range(N_CORES):
        meta = np.concatenate(
            [dstl_pc[core], nrm_pc[core], iota, bb, wt], axis=1
        ).astype(np.float32)
        in_maps.append(
            {
                "x": x_full,
                "meta": np.ascontiguousarray(meta),
                "idx": idx_pc[core],
            }
        )
    return B, in_maps



NCH = 79  # 128-node source chunks


def _prep2(x, W, b, edge_weight, edge_index):
    """Dense-cell prep: edges bucketed by (dst_tile, src_chunk) cell; the
    first 128 edges of each cell go to the main pass (one-hot + matmul),
    the rest to a small gathered leftover pass."""
    src = edge_index[0].astype(np.int64)
    dst = edge_index[1].astype(np.int64)
    ew = edge_weight.astype(np.float32)
    loops = np.arange(N_NODES, dtype=np.int64)
    src = np.concatenate([src, loops])
    dst = np.concatenate([dst, loops])
    ew = np.concatenate([ew, np.ones(N_NODES, np.float32)])

    deg = np.bincount(dst, weights=ew, minlength=N_NODES)
    dinv = np.zeros(N_NODES, np.float64)
    pos = deg > 0
    dinv[pos] = 1.0 / np.sqrt(deg[pos])
    dinv = dinv.astype(np.float32)
    norm = (dinv[src] * ew * dinv[dst]).astype(np.float32)

    g_tile = dst // P
    s_chunk = src // P
    cell = g_tile * NCH + s_chunk
    order = np.argsort(cell, kind="stable")
    src, dst, norm, cell = src[order], dst[order], norm[order], cell[order]
    g_tile = g_tile[order]

    ncells = N_TILES_TOTAL * NCH
    counts = np.bincount(cell, minlength=ncells)
    starts = np.zeros(ncells + 1, np.int64)
    np.cumsum(counts, out=starts[1:])
    q = np.arange(len(dst)) - starts[cell]

    nt = TILES_PER_CORE
    # ---- main pass: q < P ----
    main = q < P
    mslot = cell[main] * P + q[main]
    nmain = ncells * P
    srcl_f = np.zeros(nmain, np.float32)
    dstl_f = np.zeros(nmain, np.float32)
    nrm_f = np.zeros(nmain, np.float32)
    srcl_f[mslot] = (src[main] % P).astype(np.float32)
    dstl_f[mslot] = (dst[main] % P).astype(np.float32)
    nrm_f[mslot] = norm[main]
    shape = (N_CORES, nt * NCH, P)
    srcl_pc = srcl_f.reshape(shape).transpose(0, 2, 1)
    dstl_pc = dstl_f.reshape(shape).transpose(0, 2, 1)
    nrm_pc = nrm_f.reshape(shape).transpose(0, 2, 1)

    # ---- leftover pass: q >= P, grouped per (core, tile) ----
    lv = ~main
    lt_tile = g_tile[lv]  # global tile id of each leftover edge
    lt_counts = np.bincount(lt_tile, minlength=N_TILES_TOTAL)
    L = int(np.ceil(max(1, lt_counts.max()) / P)) if lt_counts.max() > 0 else 0
    lsrc = np.zeros((N_TILES_TOTAL, L * P), np.int64) if L else None
    ldst_f = np.zeros((N_TILES_TOTAL, L * P), np.float32) if L else None
    lnrm_f = np.zeros((N_TILES_TOTAL, L * P), np.float32) if L else None
    if L:
        lt_starts = np.zeros(N_TILES_TOTAL + 1, np.int64)
        np.cumsum(lt_counts, out=lt_starts[1:])
        lorder = np.argsort(lt_tile, kind="stable")
        lgt = lt_tile[lorder]
        lq = np.arange(lgt.size) - lt_starts[lgt]
        ls = src[lv][lorder]
        ld = dst[lv][lorder]
        ln = norm[lv][lorder]
        lsrc[lgt, lq] = ls
        ldst_f[lgt, lq] = (ld % P).astype(np.float32)
        lnrm_f[lgt, lq] = ln

    fp16 = bool(int(os.environ.get("GCN_FP16", "1")))
    xdt = np.float16 if fp16 else np.float32
    wt = np.ascontiguousarray(W.T.astype(np.float32))
    bb = np.tile(b.astype(np.float32)[None, :], (P, 1))
    iota = np.tile(np.arange(P, dtype=np.float32)[None, :], (P, 1))
    x_full = np.ascontiguousarray(x, dtype=xdt)
    # xres: x rows chunked so chunk sc sits at columns [sc*P:(sc+1)*P] with
    # row s on partition s: xres[p, sc*P + k] = x[sc*P + p, k]
    xpad = np.zeros((NCH * P, D), xdt)
    xpad[:N_NODES] = x_full
    xres = np.ascontiguousarray(
        xpad.reshape(NCH, P, D).transpose(1, 0, 2).reshape(P, NCH * D)
    )
    iota16 = np.tile(np.arange(P, dtype=xdt)[None, :], (P, 1))

    GC = 8
    hosts = bool(int(os.environ.get("GCN_HOSTS", "0")))
    in_maps = []
    for core in range(N_CORES):
        parts = [dstl_pc[core], nrm_pc[core], srcl_pc[core]]
        if L:
            tl = slice(core * nt, (core + 1) * nt)
            parts.append(ldst_f[tl].reshape(nt * L, P).T)
            parts.append(lnrm_f[tl].reshape(nt * L, P).T)
        parts.extend([iota, bb, wt])
        meta = np.ascontiguousarray(np.concatenate(parts, axis=1).astype(np.float32))
        m = {"meta": meta, "xres": xres, "iota16": np.ascontiguousarray(iota16)}
        if L:
            flat = lsrc[core * nt : (core + 1) * nt].reshape(-1).astype(np.int16)
            nblk = nt * L
            idx_rows = np.zeros((16, nblk * 8), np.int16)
            for sc in range((nblk + GC - 1) // GC):
                nb = min(GC, nblk - sc * GC)
                seg = flat[sc * GC * P : sc * GC * P + nb * P]
                idx_rows[:, sc * GC * 8 : sc * GC * 8 + nb * 8] = seg.reshape(
                    nb * 8, 16
                ).T
            m["idx"] = np.ascontiguousarray(np.tile(idx_rows, (8, 1)))
            m["x"] = x_full
        if hosts:
            # S one-hots [e, s] per cell, laid out [P(e), ncells*P(s)] fp16
            ncell_core = TILES_PER_CORE * NCH
            sarr = np.zeros((ncell_core, P, P), np.float16)
            srclc = srcl_pc[core]  # [P(e), ncells] float values
            nrmc = nrm_pc[core]
            e_idx, cell_idx = np.nonzero(nrmc != 0.0)
            sarr[cell_idx, e_idx, srclc[e_idx, cell_idx].astype(np.int64)] = 1.0
            m["shost"] = np.ascontiguousarray(
                sarr.transpose(1, 0, 2).reshape(P, ncell_core * P)
            )
        in_maps.append(m)
    return L, in_maps


def _build_program2(L, use_gates=True, repeat=1):
    import concourse.bacc as bacc
    import concourse.mybir as mybir
    import concourse.tile as tile
    from contextlib import ExitStack
    from concourse.tile import add_dep_helper
    from concourse import library_config

    nt = TILES_PER_CORE
    fp32 = mybir.dt.float32
    fp16 = bool(int(os.environ.get("GCN_FP16", "1")))
    dt16 = mybir.dt.float16 if fp16 else fp32
    CB = 8  # cells per C-batch (2 PSUM banks)
    GC = 8

    hosts = bool(int(os.environ.get("GCN_HOSTS", "0")))
    nc = bacc.Bacc(
        "TRN2", target_bir_lowering=False, debug=False, num_devices=N_CORES
    )
    ncols = 3 * nt * NCH + 2 * nt * L + 3 * P
    meta_d = nc.dram_tensor("meta", [P, ncols], fp32, kind="ExternalInput")
    if hosts:
        shost_d = nc.dram_tensor(
            "shost", [P, nt * NCH * P], dt16, kind="ExternalInput"
        )
    xres_d = nc.dram_tensor("xres", [P, NCH * D], dt16, kind="ExternalInput")
    iota16_d = nc.dram_tensor("iota16", [P, P], dt16, kind="ExternalInput")
    if L:
        x_d = nc.dram_tensor("x", [N_NODES, D], dt16, kind="ExternalInput")
        idx_d = nc.dram_tensor(
            "idx", [P, nt * L * 8], mybir.dt.int16, kind="ExternalInput"
        )
    out_ds = [
        nc.dram_tensor(f"out{t}", [P, D], fp32, kind="ExternalOutput")
        for t in range(nt)
    ]

    with tile.TileContext(nc) as tc, ExitStack() as ctx:
        cpool = ctx.enter_context(tc.tile_pool(name="const", bufs=1))
        mpool = ctx.enter_context(tc.tile_pool(name="onehot", bufs=16))
        cbpool = ctx.enter_context(tc.tile_pool(name="cbatch", bufs=4))
        opool = ctx.enter_context(tc.tile_pool(name="outs", bufs=12))
        ps_c = ctx.enter_context(tc.tile_pool(name="ps_c", bufs=2, space="PSUM"))
        ps_a = ctx.enter_context(tc.tile_pool(name="ps_a", bufs=2, space="PSUM"))
        ps_o = ctx.enter_context(tc.tile_pool(name="ps_o", bufs=2, space="PSUM"))

        hw_hist = []
        meta_sb = cpool.tile([P, ncols], fp32)
        meta_load = nc.sync.dma_start(meta_sb[:], meta_d[:, :])
        hw_hist.append(meta_load)
        o1 = nt * NCH
        dstl_sb = meta_sb[:, 0:o1]
        nrm_sb = meta_sb[:, o1 : 2 * o1]
        srcl_sb = meta_sb[:, 2 * o1 : 3 * o1]
        o2 = 3 * o1
        ldst_sb = meta_sb[:, o2 : o2 + nt * L]
        lnrm_sb = meta_sb[:, o2 + nt * L : o2 + 2 * nt * L]
        o3 = o2 + 2 * nt * L
        iota_sb = meta_sb[:, o3 : o3 + P]
        bb_sb = meta_sb[:, o3 + P : o3 + 2 * P]
        wt_sb = meta_sb[:, o3 + 2 * P : o3 + 3 * P]
        xres_sb = cpool.tile([P, NCH * D], dt16)
        xres_load = nc.sync.dma_start(xres_sb[:], xres_d[:, :])
        hw_hist.append(xres_load)
        iota16_sb = cpool.tile([P, P], dt16)
        hw_hist.append(nc.sync.dma_start(iota16_sb[:], iota16_d[:, :]))

        if L:
            nc.gpsimd.load_library(library_config.mlp)
            idx_sb = cpool.tile([P, nt * L * 8], mybir.dt.int16)
            hw_hist.append(nc.sync.dma_start(idx_sb[:], idx_d[:, :]))

        meta_gate = nc.tensor.nop(hint="dep")
        add_dep_helper(meta_gate.ins, meta_load.ins, reason="PE observes meta")
        xres_gate = nc.tensor.nop(hint="dep")
        add_dep_helper(xres_gate.ins, xres_load.ins, reason="PE observes xres")

        gathers = []
        if L:
            gl = cpool.tile([P, nt * L, D], dt16)
            nblk = nt * L
            for sc in range((nblk + GC - 1) // GC):
                nb = min(GC, nblk - sc * GC)
                ni = nb * P
                c0 = sc * GC * 8
                gth = nc.gpsimd.dma_gather(
                    gl[:, sc * GC : sc * GC + nb, :],
                    x_d[:, :],
                    idx_sb[:, c0 : c0 + nb * 8],
                    ni,
                    ni,
                    D,
                )
                gathers.append(gth)
                gg = nc.tensor.nop(hint="dep")
                add_dep_helper(gg.ins, gth.ins, reason="PE observes gather")

        act_hist = []
        tt_hist = []
        wmm_hist = []
        lastmm_hist = []
        ccopy_hist = []
        nbatch = (NCH + CB - 1) // CB
        for it in range(repeat * nt):
            t = it % nt
            aggT_ps = ps_a.tile([P, P], fp32)
            first_main = True
            for bt in range(nbatch):
                cells = range(bt * CB, min(NCH, (bt + 1) * CB))
                ncell = len(cells)
                gbi = it * nbatch + bt
                if gbi >= 2:
                    # absorb the C PSUM bank release (ACT copy 2 batches ago)
                    cg = nc.tensor.nop(hint="dep")
                    add_dep_helper(
                        cg.ins, ccopy_hist[gbi - 2].ins, reason="C bank free"
                    )
                C_ps = ps_c.tile([P, CB * P], fp32)
                if hosts:
                    sbatch = mpool.tile([P, CB * P], dt16, tag="S")
                    c0s = (t * NCH + bt * CB) * P
                    hw_hist.append(
                        nc.sync.dma_start(
                            sbatch[:, : ncell * P],
                            shost_d[:, c0s : c0s + ncell * P],
                        )
                    )
                for ci, sc in enumerate(cells):
                    col = t * NCH + sc
                    if hosts:
                        S = None
                    else:
                        S = mpool.tile([P, P], dt16, tag="S")
                        nc.vector.tensor_scalar(
                            S[:],
                            iota16_sb[:],
                            srcl_sb[:, col : col + 1],
                            None,
                            mybir.AluOpType.is_equal,
                        )
                    Dn = mpool.tile([P, P], dt16, tag="Dn")
                    nc.vector.tensor_scalar(
                        Dn[:],
                        iota16_sb[:],
                        dstl_sb[:, col : col + 1],
                        nrm_sb[:, col : col + 1],
                        mybir.AluOpType.is_equal,
                        mybir.AluOpType.mult,
                    )
                    nc.tensor.matmul(
                        C_ps[:, ci * P : (ci + 1) * P],
                        lhsT=(
                            sbatch[:, ci * P : (ci + 1) * P] if hosts else S[:]
                        ),
                        rhs=Dn[:],
                        start=True,
                        stop=True,
                    )
                C_sb = cbpool.tile([P, CB * P], dt16)
                ccopy = nc.scalar.activation(
                    C_sb[:, : ncell * P],
                    C_ps[:, : ncell * P],
                    mybir.ActivationFunctionType.Copy,
                )
                ccopy_hist.append(ccopy)
                for ci, sc in enumerate(cells):
                    is_last = bt == nbatch - 1 and ci == ncell - 1 and L == 0
                    mm = nc.tensor.matmul(
                        aggT_ps[:],
                        lhsT=xres_sb[:, sc * D : (sc + 1) * D],
                        rhs=C_sb[:, ci * P : (ci + 1) * P],
                        start=first_main,
                        stop=is_last,
                    )
                    first_main = False
            for l in range(L):
                colL = t * L + l
                M = mpool.tile([P, P], dt16, tag="Dn")
                nc.vector.tensor_scalar(
                    M[:],
                    iota16_sb[:],
                    ldst_sb[:, colL : colL + 1],
                    lnrm_sb[:, colL : colL + 1],
                    mybir.AluOpType.is_equal,
                    mybir.AluOpType.mult,
                )
                mm = nc.tensor.matmul(
                    aggT_ps[:],
                    lhsT=gl[:, colL, :],
                    rhs=M[:],
                    start=False,
                    stop=(l == L - 1),
                )
            lastmm_hist.append(mm)
            aggT_sb = opool.tile([P, P], fp32)
            act = nc.scalar.activation(
                aggT_sb[:], aggT_ps[:], mybir.ActivationFunctionType.Copy
            )
            act_hist.append(act)
            if it >= 2:
                og = nc.tensor.nop(hint="dep")
                add_dep_helper(
                    og.ins, tt_hist[it - 2].ins, reason="out_ps bank free"
                )
                ag = nc.tensor.nop(hint="dep")
                add_dep_helper(
                    ag.ins, act_hist[it - 2].ins, reason="aggT bank free"
                )
            out_ps = ps_o.tile([P, P], fp32)
            wmm = nc.tensor.matmul(
                out_ps[:], lhsT=aggT_sb[:], rhs=wt_sb, start=True, stop=True
            )
            wmm_hist.append(wmm)
            out_sb = opool.tile([P, D], fp32)
            tt = nc.vector.tensor_tensor(
                out_sb[:], out_ps[:], bb_sb, op=mybir.AluOpType.add
            )
            tt_hist.append(tt)
            if len(hw_hist) >= 8:
                sp_gate = nc.sync.nop(hint="dep")
                add_dep_helper(
                    sp_gate.ins, hw_hist[-8].ins, reason="HWDGE lane recycle"
                )
            hw_hist.append(nc.sync.dma_start(out_ds[t][:, :], out_sb[:]))

        tail_deps = (
            hw_hist
            + gathers
            + [tt_hist[-1], act_hist[-1], wmm_hist[-1], lastmm_hist[-1]]
            + ccopy_hist[-2:]
        )
        for dep in tail_deps:
            tn = nc.sync.nop(hint="dep")
            add_dep_helper(tn.ins, dep.ins, reason="tail drain observe")

    nc.compile()
    if use_gates:
        _dedup_waits(nc)
    return nc


def _prep3(x, W, b, edge_weight, edge_index):
    """v3 prep: direct-C cells (<=1 edge per (src, dst-tile), built on DVE by
    one fused tensor_scalar per cell) + host-pregathered overflow blocks.

    Returns (LT, in_maps) where LT = overflow blocks per tile (uniform)."""
    src = edge_index[0].astype(np.int64)
    dst = edge_index[1].astype(np.int64)
    ew = edge_weight.astype(np.float32)
    loops = np.arange(N_NODES, dtype=np.int64)
    src = np.concatenate([src, loops])
    dst = np.concatenate([dst, loops])
    ew = np.concatenate([ew, np.ones(N_NODES, np.float32)])

    deg = np.bincount(dst, weights=ew, minlength=N_NODES)
    dinv = np.zeros(N_NODES, np.float64)
    pos = deg > 0
    dinv[pos] = 1.0 / np.sqrt(deg[pos])
    dinv = dinv.astype(np.float32)
    norm = (dinv[src] * ew * dinv[dst]).astype(np.float64)

    # merge duplicate (src, dst) pairs (norms add)
    key = dst * N_NODES + src
    ukey, inv = np.unique(key, return_inverse=True)
    mnorm = np.bincount(inv, weights=norm).astype(np.float32)
    msrc = ukey % N_NODES
    mdst = ukey // N_NODES

    g = mdst // P  # global dst tile 0..78
    core = g // TILES_PER_CORE
    t = g % TILES_PER_CORE
    chunk = msrc // P
    slot = msrc % P
    dstl = mdst % P

    # layer-1: first edge per (src, gtile)
    pairkey = g * N_NODES + msrc
    order = np.argsort(pairkey, kind="stable")
    pk = pairkey[order]
    first = np.ones(pk.size, bool)
    first[1:] = pk[1:] != pk[:-1]
    l1 = order[first]
    ovf = order[~first]

    PT = int(os.environ.get("GCN_PT", "3"))  # pool-built tiles (high end)
    HT = int(os.environ.get("GCN_HT", "2"))  # host-streamed tiles (below pool)
    NDT = TILES_PER_CORE - PT  # first pool tile index
    NVT = NDT - HT  # DVE-built tiles 0..NVT-1; host tiles NVT..NDT-1
    pool_tile = t >= NDT
    host_tile = (t >= NVT) & (t < NDT)

    # meta per core: dst16/nrm16 [128(slot), NCH*TILES_PER_CORE] col = chunk*10+t
    ncells = NCH * TILES_PER_CORE
    dst16 = np.zeros((N_CORES, P, ncells), np.float32)
    nrm16 = np.zeros((N_CORES, P, ncells), np.float32)
    keep1 = ~pool_tile[l1] & ~host_tile[l1]
    l1k = l1[keep1]
    col = chunk[l1k] * TILES_PER_CORE + t[l1k]
    dst16[core[l1k], slot[l1k], col] = dstl[l1k].astype(np.float32)
    nrm16[core[l1k], slot[l1k], col] = mnorm[l1k]

    # host tiles: full dense A-blocks streamed in (all edges, incl. multi)
    hm = host_tile  # on merged edges (layer-1 + overflow alike)
    hostC = np.zeros((N_CORES, P, NCH * HT * P), np.float16)
    if HT:
        hcol = (chunk[hm] * HT + (t[hm] - NVT)) * P + dstl[hm]
        np.add.at(hostC, (core[hm], slot[hm], hcol), mnorm[hm])

    # pool tiles: ALL edges via local_scatter (idx = (t-NDT)*128+dstl)
    pe_mask = pool_tile  # every merged edge on a pool tile
    pidx_e = ((t[pe_mask] - NDT) * P + dstl[pe_mask]).astype(np.int16)
    pcore = core[pe_mask]
    pchunk = chunk[pe_mask]
    pslot = slot[pe_mask]
    pval = mnorm[pe_mask].astype(np.float16)
    # rank within (core, chunk, slot)
    pk = (pcore * NCH + pchunk) * P + pslot
    porder = np.argsort(pk, kind="stable")
    pks = pk[porder]
    prank = np.zeros(pks.size, np.int64)
    same = np.zeros(pks.size, bool)
    same[1:] = pks[1:] == pks[:-1]
    run = 0
    # vectorized rank-within-group
    starts_idx = np.flatnonzero(~same)
    grp_start = np.zeros(pks.size, np.int64)
    grp_start[starts_idx] = starts_idx
    grp_start = np.maximum.accumulate(grp_start)
    prank = np.arange(pks.size) - grp_start
    NI = int(prank.max()) + 1 if pks.size else 2
    NI += NI % 2  # even
    pidx = np.full((N_CORES, P, NCH, NI), -1, np.int16)
    pdat = np.zeros((N_CORES, P, NCH, NI), np.float16)
    po = porder
    pidx[pcore[po], pslot[po], pchunk[po], prank] = pidx_e[po]
    pdat[pcore[po], pslot[po], pchunk[po], prank] = pval[po]
    pidx = pidx.reshape(N_CORES, P, NCH * NI)
    pdat = pdat.reshape(N_CORES, P, NCH * NI)

    # overflow: per (core, tile<NVT) blocks of 128 edges
    keepo = ~pool_tile[ovf] & ~host_tile[ovf]
    ovf = ovf[keepo]
    okey = (core[ovf] * NVT + t[ovf]).astype(np.int64)
    oorder = np.argsort(okey, kind="stable")
    ov = ovf[oorder]
    ok = okey[oorder]
    counts = np.bincount(ok, minlength=N_CORES * NVT)
    LT = int(np.ceil(counts.max() / P)) if counts.max() > 0 else 1
    nblk = NVT * LT
    starts = np.zeros(N_CORES * NVT + 1, np.int64)
    np.cumsum(counts, out=starts[1:])
    rank = np.arange(ov.size) - starts[ok]

    x16 = np.ascontiguousarray(x, dtype=np.float16)
    # xovf[core][e, blk*128+k] = x[src_e, k]; ovf (dstl, nrm) go into meta
    xovf = np.zeros((N_CORES, nblk, P, D), np.float16)
    odst = np.zeros((N_CORES, P, nblk), np.float32)
    onrm = np.zeros((N_CORES, P, nblk), np.float32)
    oc = core[ov]
    ot = t[ov]
    oblk = ot * LT + rank // P
    oe = rank % P
    xovf[oc, oblk, oe] = x16[msrc[ov]]
    odst[oc, oe, oblk] = dstl[ov].astype(np.float32)
    onrm[oc, oe, oblk] = mnorm[ov]
    xovf = xovf.transpose(0, 2, 1, 3).reshape(N_CORES, P, nblk * D)

    # xres[p, c*D+k] = x[c*128+p, k]
    xpad = np.zeros((NCH * P, D), np.float16)
    xpad[:N_NODES] = x16
    xres = np.ascontiguousarray(
        xpad.reshape(NCH, P, D).transpose(1, 0, 2).reshape(P, NCH * D)
    )

    iota16 = np.tile(np.arange(P, dtype=np.float16)[None, :], (P, 1))
    wt16 = np.ascontiguousarray(W.T.astype(np.float16))  # [k, d]
    bvec = np.ascontiguousarray(b.astype(np.float32)[:, None])  # [d, 1]

    xres_full = np.concatenate([iota16, wt16, xres], axis=1).astype(np.float16)
    in_maps = []
    for c in range(N_CORES):
        d4 = dst16[c].reshape(P, NCH, TILES_PER_CORE)[:, :, :NVT]
        n4 = nrm16[c].reshape(P, NCH, TILES_PER_CORE)[:, :, :NVT]
        inter = np.empty((P, NCH, 2 * NVT), np.float32)
        inter[:, :, :NVT] = d4
        inter[:, :, NVT:] = n4
        ointer = np.empty((P, nblk, 2), np.float32)
        ointer[:, :, 0] = odst[c]
        ointer[:, :, 1] = onrm[c]
        meta = np.concatenate(
            [
                bvec,
                inter.reshape(P, 2 * NCH * NVT),
                ointer.reshape(P, 2 * nblk),
            ],
            axis=1,
        ).astype(np.float32)
        in_maps.append(
            {
                "meta": np.ascontiguousarray(meta),
                "xres": np.ascontiguousarray(xres_full),
                "xovf": np.ascontiguousarray(xovf[c]),
                "pidx": np.ascontiguousarray(pidx[c]),
                "pdat": np.ascontiguousarray(pdat[c]),
                "hostc": np.ascontiguousarray(hostC[c]),
            }
        )
    return (LT, PT, NI, HT), in_maps


def _build_program3(cfg, use_gates=True, repeat=1):
    import concourse.bacc as bacc
    import concourse.mybir as mybir
    import concourse.tile as tile
    from contextlib import ExitStack
    from concourse.tile import add_dep_helper
    from concourse import library_config

    LT, PT, NI, HT = cfg
    nt = TILES_PER_CORE
    NDT = nt - PT  # first pool tile
    NVT = NDT - HT  # DVE tiles
    fp32 = mybir.dt.float32
    fp16 = mybir.dt.float16
    i16 = mybir.dt.int16
    nblk = NVT * LT
    W_TOT = nt * P  # 1280 output cols
    NSPL = (W_TOT + 511) // 512  # 512-col splits for the tail
    CW = NVT * P  # crow dve cols
    PW = PT * P  # crow pool cols
    HW = HT * P  # host cols per chunk
    # psum layout: dve [0:CW] | host [CW:CW+HW] | pool [1024:1024+PW]
    POOL_PS = 1024
    assert CW + HW <= POOL_PS and POOL_PS + PW <= 1536

    nc = bacc.Bacc(
        "TRN2", target_bir_lowering=False, debug=False, num_devices=N_CORES
    )
    ccols = 2 * P  # iota | wt prefix inside xres
    MCOLS = 1 + 2 * NCH * NVT + 2 * nblk
    meta_d = nc.dram_tensor("meta", [P, MCOLS], fp32, kind="ExternalInput")
    xres_d = nc.dram_tensor(
        "xres", [P, ccols + NCH * D], fp16, kind="ExternalInput"
    )
    xovf_d = nc.dram_tensor("xovf", [P, nblk * D], fp16, kind="ExternalInput")
    if PT:
        pidx_d = nc.dram_tensor("pidx", [P, NCH * NI], i16, kind="ExternalInput")
        pdat_d = nc.dram_tensor("pdat", [P, NCH * NI], fp16, kind="ExternalInput")
    if HT:
        hostc_d = nc.dram_tensor(
            "hostc", [P, NCH * HT * P], fp16, kind="ExternalInput"
        )
    out_d = nc.dram_tensor("out", [P, W_TOT], fp32, kind="ExternalOutput")

    XSPL = 6  # xres load split
    MSPL = 4  # meta load split
    OSPL = 4  # xovf load split
    ACT_CELLS_A = int(os.environ.get("GCN_ACT_A", "0"))
    ACT_CELLS_B = int(os.environ.get("GCN_ACT_B", "0"))
    NSEC = 3 if (ACT_CELLS_A or ACT_CELLS_B) else 2  # meta sections to load
    OVF_START = int(os.environ.get("GCN_OVS", "30"))
    OVF_END = int(os.environ.get("GCN_OVE", "77"))

    with tile.TileContext(nc) as tc, ExitStack() as ctx:
        cpool = ctx.enter_context(tc.tile_pool(name="const", bufs=1))
        CROW = int(os.environ.get("GCN_CROW", "6"))
        crow = ctx.enter_context(tc.tile_pool(name="crow", bufs=CROW))
        mpool = ctx.enter_context(tc.tile_pool(name="movf", bufs=64))
        opool = ctx.enter_context(tc.tile_pool(name="outs", bufs=2))
        spool = ctx.enter_context(tc.tile_pool(name="scratch", bufs=3))
        ps_a = ctx.enter_context(tc.tile_pool(name="ps_a", bufs=2, space="PSUM"))
        ps_o = ctx.enter_context(tc.tile_pool(name="ps_o", bufs=2, space="PSUM"))

        hw_hist = []

        def dma(dst_ap, src_ap, eng=None):
            eng = eng or nc.sync
            if len(hw_hist) >= 8:
                lane_gate = eng.nop(hint="dep")
                add_dep_helper(
                    lane_gate.ins, hw_hist[-8].ins, reason="DMA lane recycle"
                )
            h = eng.dma_start(dst_ap, src_ap)
            hw_hist.append(h)
            return h

        meta_sb = cpool.tile([P, MCOLS], fp32)
        bvec_sb = meta_sb[:, 0:1]
        OBASE = 1 + 2 * NCH * NVT

        def odst_col(b):
            return meta_sb[:, OBASE + 2 * b : OBASE + 2 * b + 1]

        def onrm_col(b):
            return meta_sb[:, OBASE + 2 * b + 1 : OBASE + 2 * b + 2]

        def dst_col(c, tt):
            base = 1 + c * 2 * NVT + tt
            return meta_sb[:, base : base + 1]

        def nrm_col(c, tt):
            base = 1 + c * 2 * NVT + NVT + tt
            return meta_sb[:, base : base + 1]
        xres_all = cpool.tile([P, ccols + NCH * D], fp16)
        iota_sb = xres_all[:, 0:P]
        wt_sb = xres_all[:, P : 2 * P]
        xres_sb = xres_all[:, ccols : ccols + NCH * D]
        xovf_sb = cpool.tile([P, nblk * D], fp16)
        if PT:
            nc.gpsimd.load_library(library_config.local_scatter)
            pidx_sb = cpool.tile([P, NCH * NI], i16)
            pdat_sb = cpool.tile([P, NCH * NI], fp16)
        if HT:
            hostc_sb = cpool.tile([P, NCH * HT * P], fp16)

        # interleaved load order: meta_q0, xres_q0 first so compute starts
        # early; overflow pieces spread between the later xres quarters.
        # progressive piece sizes: small first pieces for startup latency
        XPIECES = [8, 12, 20, 19, 20]
        MPIECES = [12, 16, 21, 30]
        xbounds = [0]
        for n in XPIECES:
            xbounds.append(min(NCH, xbounds[-1] + n))
        mbounds = [0]
        for n in MPIECES:
            mbounds.append(min(NCH, mbounds[-1] + n))
        meta_loads = []  # one handle per piece
        xres_loads = []
        ovf_loads = []  # (first_blk_covered_exclusive_end, handle)
        nbo = (nblk + OSPL - 1) // OSPL  # blocks per ovf piece

        def load_meta_q(q):
            c0, c1 = mbounds[q], mbounds[q + 1]
            a = 1 + c0 * 2 * NVT
            b = 1 + c1 * 2 * NVT
            if q == 0:
                a = 0
            meta_loads.append(dma(meta_sb[:, a:b], meta_d[:, a:b]))

        hostc_loads = {}

        def load_hostc_q(q):
            if not HT:
                return
            c0, c1 = xbounds[q], xbounds[q + 1]
            hostc_loads[q] = dma(
                hostc_sb[:, c0 * HW : c1 * HW],
                hostc_d[:, c0 * HW : c1 * HW],
            )

        def load_xres_q(q):
            c0, c1 = xbounds[q], xbounds[q + 1]
            a = ccols + c0 * D
            b = ccols + c1 * D
            xres_loads.append(dma(xres_all[:, a:b], xres_d[:, a:b]))

        def load_ovf_piece(i):
            b0, b1 = i * nbo, min(nblk, (i + 1) * nbo)
            if b0 >= b1:
                return
            hx = dma(xovf_sb[:, b0 * D : b1 * D], xovf_d[:, b0 * D : b1 * D])
            ovf_loads.append((b1, hx, hx))

        # iota|wt prefix first so DVE unblocks early
        xpre_load = dma(xres_all[:, 0:ccols], xres_d[:, 0:ccols])
        load_meta_q(0)
        load_xres_q(0)
        ovfmeta_load = dma(
            meta_sb[:, OBASE : OBASE + 2 * nblk],
            meta_d[:, OBASE : OBASE + 2 * nblk],
        )
        ph = 40
        if PT:
            pidx_loads = [
                dma(pidx_sb[:, : ph * NI], pidx_d[:, : ph * NI]),
            ]
            pdat_loads = [
                dma(pdat_sb[:, : ph * NI], pdat_d[:, : ph * NI]),
            ]
        load_hostc_q(0)
        load_xres_q(1)
        load_hostc_q(1)
        load_meta_q(1)
        load_xres_q(2)
        load_hostc_q(2)
        load_meta_q(2)
        load_ovf_piece(0)
        if PT:
            pidx_loads.append(dma(pidx_sb[:, ph * NI :], pidx_d[:, ph * NI :]))
            pdat_loads.append(dma(pdat_sb[:, ph * NI :], pdat_d[:, ph * NI :]))
        load_xres_q(3)
        load_hostc_q(3)
        load_meta_q(3)
        load_ovf_piece(1)
        load_xres_q(4)
        load_hostc_q(4)
        load_ovf_piece(2)
        load_ovf_piece(3)

        # PE/DVE observe the iota|wt prefix once
        meta_gate = nc.tensor.nop(hint="dep")
        add_dep_helper(meta_gate.ins, xpre_load.ins, reason="PE sees const")
        cg = nc.vector.nop(hint="dep")
        add_dep_helper(cg.ins, xpre_load.ins, reason="DVE sees const")
        # preload the ACT function table off the critical tail
        ag0 = nc.scalar.nop(hint="dep")
        add_dep_helper(ag0.ins, xpre_load.ins, reason="ACT sees const")
        warm = spool.tile([P, 1], fp16)
        nc.scalar.activation(
            warm[:], xres_all[:, 0:1], mybir.ActivationFunctionType.Copy
        )
        if PT:
            pg1 = nc.gpsimd.nop(hint="dep")
            add_dep_helper(pg1.ins, pidx_loads[0].ins, reason="Pool pidx a")
            pg2 = nc.gpsimd.nop(hint="dep")
            add_dep_helper(pg2.ins, pdat_loads[0].ins, reason="Pool pdat a")

        # overflow mm schedule: block -> chunk, spread over the window
        nwin = OVF_END - OVF_START + 1
        ovf_sched = {}
        cons_chunk = {}
        for b in range(nblk):
            c = OVF_START + b * nwin // nblk
            ovf_sched.setdefault(c, []).append(b)
            cons_chunk[b] = c
        # M one-hot build schedule: first EM blocks built in the pre-window
        # DVE slack (chunks MB0..OVF_START-2), the rest at consumption-2
        EM = min(nblk, int(os.environ.get("GCN_EM", "0")))
        MB0 = 4
        mb_sched = {}
        for b in range(nblk):
            if b < EM:
                c = MB0 + b * (OVF_START - 2 - MB0) // EM
            else:
                c = cons_chunk[b] - 2
            c = min(c, cons_chunk[b] - 1)
            mb_sched.setdefault(c, []).append(b)


        ts_hist = []
        mm_hist = []  # last chunk-mm per chunk
        act_hist = []
        bias_hist = []
        wmm_hist = []
        out_dmas = []
        last_mm = None
        for rep in range(repeat):
            aggps = ps_a.tile([P, 1536], fp32)  # 3 banks; cols 0:1280 used
            if rep >= 2:
                rg = nc.tensor.nop(hint="dep")
                add_dep_helper(
                    rg.ins, act_hist[rep - 2].ins, reason="aggps bank free"
                )
            ovf_dma_gated = 0
            m_tiles = {}
            for c in range(NCH):
                gi = rep * NCH + c
                if gi >= CROW:
                    # crow slot release: DVE/Pool wait PE done reading
                    vg = nc.vector.nop(hint="dep")
                    add_dep_helper(
                        vg.ins, mm_hist[gi - CROW].ins, reason="crow slot free"
                    )
                    if PT:
                        pvg = nc.gpsimd.nop(hint="dep")
                        add_dep_helper(
                            pvg.ins, mm_hist[gi - CROW].ins, reason="crow free P"
                        )

                if rep == 0 and c in mbounds[:-1]:
                    q = mbounds.index(c)
                    vg1 = nc.vector.nop(hint="dep")
                    add_dep_helper(
                        vg1.ins, meta_loads[q].ins, reason="DVE sees meta q"
                    )
                if rep == 0 and PT and c in (0, 40):
                    pi = 0 if c == 0 else 1
                    pg3 = nc.gpsimd.nop(hint="dep")
                    add_dep_helper(
                        pg3.ins, pidx_loads[pi].ins, reason="Pool pidx q"
                    )
                    pg4 = nc.gpsimd.nop(hint="dep")
                    add_dep_helper(
                        pg4.ins, pdat_loads[pi].ins, reason="Pool pdat q"
                    )
                C = crow.tile([P, CW + PW], fp16)
                if PT:
                    pls = nc.gpsimd.local_scatter(
                        C[:, CW : CW + PW],
                        pdat_sb[:, c * NI : (c + 1) * NI],
                        pidx_sb[:, c * NI : (c + 1) * NI],
                        P,
                        PW,
                        NI,
